# revision 42
# baseline (speedup 1.0000x reference)
"""Trainium2 (Bass/Tile) kernel for quantized multi-head attention.

Distributed across 8 NeuronCores: tensor-parallel over heads for the
QKV projections + RoPE + causal attention, per-batch AllToAll
collectives (overlapped with later batches' compute), then a
token-parallel output projection over interleaved 128-token tiles.

The Q4_0 weights ship host-UNPACKED (int4 values widened to int8, laid
out transposed [in%128, in//128, out] so dequant lands directly in the
matmul-rhs layout with no PE transposes) alongside host-expanded group
scales; on-chip dequant is a single chunked tensor_tensor multiply (DVE
for QKV at startup, GpSimd for the wo panels so they overlap the last
batch's attention). The causal mask is a 0/1 multiply after exp, and
softmax normalization divides after the PV matmul (linearity), with the
1/z partition-broadcast on GpSimd.
"""

import math
from dataclasses import dataclass

import numpy as np

import concourse.bass as bass
import concourse.tile as tile
from concourse.masks import make_identity
from concourse import bacc, mybir

BF = mybir.dt.bfloat16
F32 = mybir.dt.float32
I8 = mybir.dt.int8
AOP = mybir.AluOpType
AF = mybir.ActivationFunctionType


@dataclass
class Cfg:
    B: int = 4
    S: int = 1024
    D: int = 4096
    NCORES: int = 8
    SCH: int = 512   # kept for test.py compat (unused)
    QCH: int = 512   # attention q-chunk

    @property
    def T(self):
        return self.B * self.S

    @property
    def H(self):
        return self.D // 128  # total heads (head_dim 128)

    @property
    def H_LOC(self):
        return self.H // self.NCORES

    @property
    def C_SHARD(self):
        return self.H_LOC * 128  # local channels

    @property
    def TPC(self):
        return self.T // self.NCORES  # tokens per core (output slice)

    @property
    def NGP(self):
        return self.D // 128  # contraction k-tiles per row


def build_program(cfg: Cfg):
    """Build the per-core Bass program. Returns compiled nc."""
    c = cfg
    assert c.QCH == 512 and c.S == 1024 and c.NCORES == 8

    import concourse.tile_utils as tile_utils
    tile_utils.max_sbuf_usage = 208 * 1024

    nc = bacc.Bacc("TRN2", target_bir_lowering=False, debug=False,
                   num_devices=c.NCORES)

    OSH = c.C_SHARD          # qkv weight shard out-channels per core (512)
    NGP = c.NGP              # 32
    NTIL = c.T // 128        # 32 global token tiles
    TPB = c.S // 128         # 8 tiles per batch

    # ---- external I/O ----
    # x retiled: [p=i%128, tile, g=i//128, t']
    x_d = nc.dram_tensor("x", [128, NTIL, NGP, 128], BF, kind="ExternalInput")
    # unpacked int4 values, transposed: wt[p=i%128, g=i//128, o]
    w_q = nc.dram_tensor("wq_w", [128, NGP, OSH], I8, kind="ExternalInput")
    s_q = nc.dram_tensor("wq_s", [128, NGP, OSH], BF, kind="ExternalInput")
    w_k = nc.dram_tensor("wk_w", [128, NGP, OSH], I8, kind="ExternalInput")
    s_k = nc.dram_tensor("wk_s", [128, NGP, OSH], BF, kind="ExternalInput")
    w_v = nc.dram_tensor("wv_w", [128, NGP, OSH], I8, kind="ExternalInput")
    s_v = nc.dram_tensor("wv_s", [128, NGP, OSH], BF, kind="ExternalInput")
    # wo panel-major: [p, oc, g, o']
    w_o = nc.dram_tensor("wo_w", [128, c.D // 512, NGP, 512], I8,
                         kind="ExternalInput")
    s_o = nc.dram_tensor("wo_s", [128, c.D // 512, NGP, 512], BF,
                         kind="ExternalInput")
    # rope tables, compact: [p=s%128, ssub=s//128, d]
    cosc_d = nc.dram_tensor("cosc", [128, TPB, 128], BF, kind="ExternalInput")
    sinc_d = nc.dram_tensor("sinc", [128, TPB, 128], BF, kind="ExternalInput")
    maskd_d = nc.dram_tensor("maskd", [128, 128], BF, kind="ExternalInput")
    out_d = nc.dram_tensor("out", [c.TPC, c.D], BF, kind="ExternalOutput")

    # per-batch collective bounce buffers; slot j = within-batch token tile j
    a2a_in = [nc.dram_tensor(f"a2a_in{b}", [c.NCORES, c.C_SHARD, 128], BF)
              for b in range(c.B)]
    a2a_out = [nc.dram_tensor(f"a2a_out{b}", [c.NCORES, c.C_SHARD, 128], BF)
               for b in range(c.B)]

    inv_sqrt_d = 1.0 / math.sqrt(128.0)

    def dequant_t(pool, wt, bt_ap, sc_ap, ngp, osz, chunks=4, eng=None):
        """Dequantize unpacked int4 values into transposed wt [128, ngp, osz].

        bt_ap: DRAM [128, ngp, osz] int8 values; sc_ap: DRAM [128, ngp, osz]
        host-expanded scales. Works in double-buffered [128, ngp/chunks, osz]
        chunk tiles so DMAs pipeline and consumers can start early."""
        if eng is None:
            eng = nc.vector
        gch = ngp // chunks
        for i in range(chunks):
            g0 = i * gch
            nq = pool.tile([128, gch, osz], I8, tag="dq_nb", bufs=2)
            sc = pool.tile([128, gch, osz], BF, tag="dq_sc", bufs=2)
            nc.sync.dma_start(nq[:], bt_ap[:, g0:g0 + gch, :])
            nc.sync.dma_start(sc[:], sc_ap[:, g0:g0 + gch, :])
            eng.tensor_tensor(
                out=wt[:, g0:g0 + gch, :], in0=nq[:], in1=sc[:],
                op=AOP.mult)

    with tile.TileContext(nc) as tc:
        with tc.tile_pool(name="const", bufs=1) as const, \
             tc.tile_pool(name="sbuf", bufs=2) as sbuf:
            # constants
            cosc = const.tile([128, TPB, 128], BF)
            nc.sync.dma_start(cosc[:], cosc_d[:])
            sinc = const.tile([128, TPB, 128], BF)
            nc.sync.dma_start(sinc[:], sinc_d[:])
            maskd = const.tile([128, 128], BF)
            nc.sync.dma_start(maskd[:], maskd_d[:])
            ones_col = const.tile([128, 1], BF)
            nc.vector.memset(ones_col[:], 1.0)
            ones_row = const.tile([1, 128], BF)
            nc.vector.memset(ones_row[:], 1.0)
            ident = const.tile([128, 128], BF)
            make_identity(nc, ident)

            # ============ phase 1: QKV + attention ============
            with tc.tile_pool(name="wtq", bufs=1) as wtqp, \
                 tc.tile_pool(name="wt", bufs=1) as wtp:
                wt_q = wtqp.tile([128, NGP, OSH], BF, tag="wt_q")
                wt_k = wtp.tile([128, NGP, OSH], BF, tag="wt_k")
                wt_v = wtp.tile([128, NGP, OSH], BF, tag="wt_v")
                with tc.tile_pool(name="dqp", bufs=1) as dqp, \
                     tc.tile_pool(name="xt", bufs=1) as xtp, \
                     tc.tile_pool(name="kqv", bufs=2) as kqvp, \
                     tc.tile_pool(name="pt", bufs=4) as ptp, \
                     tc.tile_pool(name="ppsum", bufs=2, space="PSUM") as ppsum, \
                     tc.tile_pool(name="spsum", bufs=2, space="PSUM") as spsum, \
                     tc.tile_pool(name="zpsum", bufs=1, space="PSUM") as zpsum, \
                     tc.tile_pool(name="apsum", bufs=1, space="PSUM") as apsum, \
                     tc.tile_pool(name="tpsum", bufs=2, space="PSUM") as tpsum:

                    def proj_one(mat, wt_m, xt_ts, ts, kt_b, qt_b, v_b):
                        st0 = ts * 128
                        ps = ppsum.tile([128, OSH], F32, tag="proj")
                        for gp in range(NGP):
                            nc.tensor.matmul(
                                ps[:],
                                lhsT=xt_ts[:, gp, :],
                                rhs=wt_m[:, gp, :],
                                start=(gp == 0),
                                stop=(gp == NGP - 1))
                        if mat == "v":
                            nc.scalar.copy(out=v_b[:, ts, :], in_=ps[:])
                            return
                        # single PSUM read, then rope from SBUF bf16
                        psc = sbuf.tile([128, c.C_SHARD], BF,
                                        tag="psc", bufs=2)
                        nc.scalar.copy(out=psc[:], in_=ps[:])
                        # rope: roped = psc*cos + swaphalf(psc)*sin(+/-)
                        roped = sbuf.tile([128, c.C_SHARD], BF,
                                          tag="roped", bufs=2)
                        tmp = sbuf.tile([128, c.C_SHARD], BF,
                                        tag="ropetmp", bufs=2)
                        p3 = psc[:].rearrange("p (h d) -> p h d", d=128)
                        t3 = tmp[:].rearrange("p (h d) -> p h d", d=128)
                        r3 = roped[:].rearrange("p (h d) -> p h d", d=128)
                        c3 = cosc[:, ts, :][:, None, :].to_broadcast(
                            [128, c.H_LOC, 128])
                        s3 = sinc[:, ts, :][:, None, :].to_broadcast(
                            [128, c.H_LOC, 128])
                        nc.vector.tensor_tensor(
                            out=t3[:, :, 0:64], in0=p3[:, :, 64:128],
                            in1=s3[:, :, 0:64], op=AOP.mult)
                        nc.vector.tensor_tensor(
                            out=t3[:, :, 64:128], in0=p3[:, :, 0:64],
                            in1=s3[:, :, 64:128], op=AOP.mult)
                        nc.vector.tensor_tensor(
                            out=r3[:], in0=p3[:], in1=c3, op=AOP.mult)
                        nc.vector.tensor_tensor(
                            out=roped[:], in0=roped[:], in1=tmp[:],
                            op=AOP.add)
                        dst = qt_b if mat == "q" else kt_b
                        for h in range(c.H_LOC):
                            tp = tpsum.tile([128, 128], BF, tag="tp",
                                            bufs=2)
                            nc.tensor.transpose(
                                tp[:], roped[:, h * 128:(h + 1) * 128],
                                ident[:])
                            if h % 2 == 0:
                                nc.scalar.copy(
                                    out=dst[:, h, st0:st0 + 128], in_=tp[:])
                            else:
                                nc.vector.tensor_copy(
                                    out=dst[:, h, st0:st0 + 128], in_=tp[:])

                    dq_srcs = {"q": (wt_q, w_q, s_q), "k": (wt_k, w_k, s_k),
                               "v": (wt_v, w_v, s_v)}
                    for b in range(c.B):
                        # per-batch K/Q transposed and V natural
                        kt_b = kqvp.tile([128, c.H_LOC, c.S], BF, tag="kt_b")
                        qt_b = kqvp.tile([128, c.H_LOC, c.S], BF, tag="qt_b")
                        v_b = kqvp.tile([128, TPB, c.C_SHARD], BF, tag="v_b",
                                        bufs=1)
                        if b == 0:
                            # mat-outer, dequant interleaved: q projections
                            # start right after wq's dequant while k/v still
                            # dequantize (x tiles re-loaded per mat)
                            for mat, wt_m in (("q", wt_q), ("k", wt_k),
                                              ("v", wt_v)):
                                _, w_m, s_m = dq_srcs[mat]
                                dequant_t(dqp, wt_m, w_m.ap(), s_m.ap(),
                                          NGP, OSH, chunks=8)
                                for ts in range(TPB):
                                    xt_ts = xtp.tile([128, NGP, 128], BF,
                                                     tag="xt", bufs=2)
                                    nc.sync.dma_start(xt_ts[:],
                                                      x_d.ap()[:, ts])
                                    proj_one(mat, wt_m, xt_ts, ts,
                                             kt_b, qt_b, v_b)
                        else:
                            for ts in range(TPB):
                                tt = b * TPB + ts
                                xt_ts = xtp.tile([128, NGP, 128], BF,
                                                 tag="xt", bufs=2)
                                nc.sync.dma_start(xt_ts[:], x_d.ap()[:, tt])
                                for mat, wt_m in (("q", wt_q), ("k", wt_k),
                                                  ("v", wt_v)):
                                    proj_one(mat, wt_m, xt_ts, ts,
                                             kt_b, qt_b, v_b)

                        if b == c.B - 1:
                            for oc, wt_m in enumerate((wt_q, wt_k, wt_v)):
                                dequant_t(dqp, wt_m, w_o.ap()[:, oc],
                                          s_o.ap()[:, oc], NGP, 512,
                                          chunks=8, eng=nc.gpsimd)

                        # ---- attention for batch b ----
                        for h in range(c.H_LOC):
                            for qc in range(c.S // c.QCH):
                                q0 = qc * c.QCH
                                kmax = (q0 + c.QCH) // 128
                                at = apsum.tile([128, c.QCH], F32, tag="at")
                                zp = zpsum.tile([1, c.QCH], F32, tag="z")
                                psum_tree = sbuf.tile([128, c.QCH], BF,
                                                      tag="ptree", bufs=2)
                                for ki in range(kmax):
                                    off = max(0, 128 * ki - q0)
                                    stp = spsum.tile([128, c.QCH], F32,
                                                     tag="sc")
                                    nc.tensor.matmul(
                                        stp[:, off:],
                                        lhsT=kt_b[:, h,
                                                  ki * 128:(ki + 1) * 128],
                                        rhs=qt_b[:, h, q0 + off:q0 + c.QCH],
                                        start=True, stop=True)
                                    pt = ptp.tile([128, c.QCH], BF, tag="pt")
                                    nc.scalar.activation(
                                        out=pt[:, off:], in_=stp[:, off:],
                                        func=AF.Exp, scale=inv_sqrt_d)
                                    if 128 * ki >= q0:
                                        # zero the upper triangle of the
                                        # diagonal block (causal mask)
                                        nc.vector.tensor_tensor(
                                            out=pt[:, off:off + 128],
                                            in0=pt[:, off:off + 128],
                                            in1=maskd[:], op=AOP.mult)
                                    if ki == 0:
                                        nc.vector.tensor_copy(
                                            out=psum_tree[:], in_=pt[:])
                                    else:
                                        nc.vector.tensor_tensor(
                                            out=psum_tree[:, off:],
                                            in0=psum_tree[:, off:],
                                            in1=pt[:, off:], op=AOP.add)
                                    nc.tensor.matmul(
                                        at[:, off:],
                                        lhsT=v_b[:, ki,
                                                 h * 128:(h + 1) * 128],
                                        rhs=pt[:, off:],
                                        start=(ki == 0),
                                        stop=(ki == kmax - 1))
                                nc.tensor.matmul(
                                    zp[:], lhsT=ones_col[:], rhs=psum_tree[:],
                                    start=True, stop=True)
                                rzf = sbuf.tile([1, c.QCH], F32, tag="rzf")
                                nc.vector.reciprocal_approx_fast(rzf[:],
                                                                 zp[:])
                                rz = sbuf.tile([1, c.QCH], BF, tag="rz")
                                nc.vector.tensor_copy(out=rz[:], in_=rzf[:])
                                bzp = spsum.tile([128, c.QCH], F32, tag="sc")
                                nc.tensor.matmul(
                                    bzp[:], lhsT=ones_row[:], rhs=rz[:],
                                    start=True, stop=True)
                                # only one PSUM input allowed per DVE op:
                                # drain the unnormalized output to SBUF first
                                aosb = sbuf.tile([128, c.QCH], BF,
                                                 tag="aosb", bufs=2)
                                nc.scalar.copy(out=aosb[:], in_=at[:])
                                ao = sbuf.tile([128, c.QCH], BF, tag="ao")
                                nc.vector.tensor_tensor(
                                    out=ao[:], in0=aosb[:], in1=bzp[:],
                                    op=AOP.mult)
                                for j in range(c.QCH // 128):
                                    slot = qc * (c.QCH // 128) + j
                                    nc.sync.dma_start(
                                        out=a2a_in[b][slot][
                                            h * 128:(h + 1) * 128, :],
                                        in_=ao[:, j * 128:(j + 1) * 128])

                        # per-batch collective, overlaps next batch's compute
                        nc.gpsimd.collective_compute(
                            "AllToAll", AOP.bypass,
                            replica_groups=[list(range(c.NCORES))],
                            ins=[a2a_in[b].ap().opt()],
                            outs=[a2a_out[b].ap().opt()],
                        )

                # ===== phase 2: output projection (token-sharded) =====
                with tc.tile_pool(name="gath", bufs=1) as gathp, \
                     tc.tile_pool(name="p2", bufs=1) as p2p, \
                     tc.tile_pool(name="wpsum", bufs=2, space="PSUM") as wpsum:
                    gaths = []
                    for b in range(c.B):
                        g = gathp.tile([128, NGP, 128], BF, tag=f"gath{b}")
                        nc.sync.dma_start(
                            g[:],
                            a2a_out[b].ap().rearrange(
                                "r (g p) t -> p (r g) t", p=128))
                        gaths.append(g)

                    def wo_gemm(oc, b, panel):
                        ops = wpsum.tile([128, 512], F32, tag="wo")
                        for ct in range(NGP):
                            nc.tensor.matmul(
                                ops[:], lhsT=gaths[b][:, ct, :],
                                rhs=panel[:, ct, :],
                                start=(ct == 0), stop=(ct == NGP - 1))
                        osb = sbuf.tile([128, 512], BF, tag="osb", bufs=2)
                        nc.scalar.copy(out=osb[:], in_=ops[:])
                        nc.sync.dma_start(
                            out=out_d[b * 128:(b + 1) * 128,
                                      oc * 512:(oc + 1) * 512],
                            in_=osb[:])

                    DEFER = 2
                    NOC = c.D // 512
                    rot = (wt_q, wt_k, wt_v)
                    for oc in range(NOC + DEFER):
                        src_oc = oc if oc < NOC else oc - NOC
                        panel = rot[oc % 3]
                        if oc >= 3:
                            dequant_t(p2p, panel, w_o.ap()[:, src_oc],
                                      s_o.ap()[:, src_oc], NGP, 512,
                                      chunks=4)
                        if oc < NOC:
                            nb = c.B - 1 if oc < DEFER else c.B
                            for b in range(nb):
                                wo_gemm(oc, b, panel)
                        else:
                            wo_gemm(src_oc, c.B - 1, panel)

    nc.compile()
    return nc


# ---------------- host-side input prep ----------------

def prep_core_inputs(cfg: Cfg, x, cos_half, sin_half, mask,
                     wq_w, wq_s, wk_w, wk_s, wv_w, wv_s, wo_w, wo_s):
    """Build in_maps (list of dicts, one per core) from full inputs."""
    import ml_dtypes
    c = cfg
    bf16 = ml_dtypes.bfloat16
    HD2 = 64
    NGP = c.NGP
    OSH = c.C_SHARD
    TPB = c.S // 128

    # x retiled: [p=i%128, tile, g=i//128, t']
    x5 = np.ascontiguousarray(
        np.asarray(x).reshape(c.T // 128, 128, NGP, 128).transpose(3, 0, 2, 1)
    ).astype(bf16, copy=False)

    # rope tables [128, TPB, 128], compact (broadcast over heads on-chip)
    ch = np.asarray(cos_half, np.float32)  # [S, 64]
    sh = np.asarray(sin_half, np.float32)
    cos = np.concatenate([ch, ch], axis=1).astype(bf16).astype(np.float32)
    sin = np.concatenate([sh, sh], axis=1).astype(bf16).astype(np.float32)
    sins = sin.copy()
    sins[:, :HD2] = -sin[:, :HD2]
    cosc = np.ascontiguousarray(
        cos.reshape(TPB, 128, 128).transpose(1, 0, 2)).astype(bf16)
    sinc = np.ascontiguousarray(
        sins.reshape(TPB, 128, 128).transpose(1, 0, 2)).astype(bf16)

    # diagonal 0/1 mask block: maskd[k, q] = 1 where mask[q, k] == 0
    m = np.asarray(mask, np.float32)[:128, :128]
    maskd = (m.T == 0.0).astype(np.float32).astype(bf16)

    def pack_w(pw, o_n, panel=None):
        """packed [o_n*NGP, 64] -> unpacked int4 values [128, NGP, o_n]
        with w[p, g, o] = W_q[o, 128*g + p] (or panel-major 4D)."""
        a = np.asarray(pw).reshape(o_n, NGP, 64)
        msb = (a >> 4).astype(np.int8)                    # i = 128g + f
        lsb = (((a & 15) ^ 8) - 8).astype(np.int8)        # i = 128g + 64 + f
        full = np.concatenate(
            [msb.transpose(2, 1, 0), lsb.transpose(2, 1, 0)], axis=0)
        if panel is None:
            return np.ascontiguousarray(full)             # [128, NGP, o_n]
        full = full.reshape(128, NGP, panel, o_n // panel)
        return np.ascontiguousarray(full.transpose(0, 2, 1, 3))

    def pack_s(ps, o_n, panel=None):
        """scales [o_n*2*NGP, 1] -> host-expanded [128, NGP, o_n]
        (rows 0:64 msb scale, 64:128 lsb scale), or panel-major 4D."""
        a = np.asarray(ps).astype(np.float32).reshape(o_n, NGP, 2)
        two = a.transpose(2, 1, 0)  # [2, NGP, o_n]
        full = np.concatenate([
            np.broadcast_to(two[0:1], (64, NGP, o_n)),
            np.broadcast_to(two[1:2], (64, NGP, o_n))], axis=0)
        if panel is None:
            return np.ascontiguousarray(full).astype(bf16)
        full = full.reshape(128, NGP, panel, o_n // panel)
        return np.ascontiguousarray(full.transpose(0, 2, 1, 3)).astype(bf16)

    wo_bt = pack_w(wo_w, c.D, panel=c.D // 512)
    wo_sc = pack_s(wo_s, c.D, panel=c.D // 512)

    in_maps = []
    for core in range(c.NCORES):
        r0 = core * OSH * NGP
        g0 = core * OSH * 2 * NGP
        in_maps.append({
            "x": x5,
            "wq_w": pack_w(np.asarray(wq_w)[r0:r0 + OSH * NGP], OSH),
            "wq_s": pack_s(np.asarray(wq_s)[g0:g0 + OSH * 2 * NGP], OSH),
            "wk_w": pack_w(np.asarray(wk_w)[r0:r0 + OSH * NGP], OSH),
            "wk_s": pack_s(np.asarray(wk_s)[g0:g0 + OSH * 2 * NGP], OSH),
            "wv_w": pack_w(np.asarray(wv_w)[r0:r0 + OSH * NGP], OSH),
            "wv_s": pack_s(np.asarray(wv_s)[g0:g0 + OSH * 2 * NGP], OSH),
            "wo_w": wo_bt,
            "wo_s": wo_sc,
            "cosc": cosc,
            "sinc": sinc,
            "maskd": maskd,
        })
    return in_maps


def unshard_output(cfg: Cfg, results):
    """results: list per core of {"out": [TPC, D]}. Returns [B, S, D].

    Core j's output rows b*128:(b+1)*128 hold global token tile 8*b + j."""
    c = cfg
    TPB = c.S // 128
    full = np.empty((c.B * TPB, 128, c.D),
                    dtype=np.asarray(results[0]["out"]).dtype)
    for j in range(c.NCORES):
        o = np.asarray(results[j]["out"]).reshape(c.B, 128, c.D)
        for b in range(c.B):
            full[TPB * b + j] = o[b]
    return full.reshape(c.B, c.S, c.D)


# ======================================================================
# Self-contained kernel entry point.
# ======================================================================

_CACHE = {}


def _get_program(cfg):
    key = (cfg.B, cfg.S, cfg.D, cfg.NCORES, cfg.SCH, cfg.QCH)
    if key not in _CACHE:
        _CACHE[key] = build_program(cfg)
    return _CACHE[key]


def kernel(x, start_pos=0, cos_half=None, sin_half=None, mask=None,
           wq_w=None, wq_s=None, wk_w=None, wk_s=None,
           wv_w=None, wv_s=None, wo_w=None, wo_s=None,
           cache_k_w=None, cache_k_s=None, cache_v_w=None, cache_v_s=None,
           **_unused):
    from concourse.bass_utils import run_bass_kernel_spmd

    assert int(start_pos) == 0, "kernel specialised for start_pos == 0"
    x = np.asarray(x)
    B, S, D = x.shape
    cfg = Cfg(B=B, S=S, D=D, NCORES=8, SCH=512, QCH=512)
    # start_pos==0 with S==MAX_S, B==MAX_B: the quantized KV cache is fully
    # overwritten before use, so cache_* inputs cannot affect the output.
    in_maps = prep_core_inputs(cfg, x, cos_half, sin_half, mask,
                               wq_w, wq_s, wk_w, wk_s, wv_w, wv_s,
                               wo_w, wo_s)
    nc = _get_program(cfg)
    res = run_bass_kernel_spmd(nc, in_maps, core_ids=list(range(cfg.NCORES)))
    out = unshard_output(cfg, res.results)
    import ml_dtypes
    return out.astype(ml_dtypes.bfloat16, copy=False)


# revision 46
# speedup vs baseline: 1.0202x; 1.0202x over previous
"""Trainium2 (Bass/Tile) kernel for quantized multi-head attention.

Distributed across 8 NeuronCores: tensor-parallel over heads for the
QKV projections + RoPE + causal attention, per-batch AllToAll
collectives (overlapped with later batches' compute), then a
token-parallel output projection over interleaved 128-token tiles.

The Q4_0 weights ship host-UNPACKED (int4 values widened to int8, laid
out transposed [in%128, in//128, out] so dequant lands directly in the
matmul-rhs layout with no PE transposes) alongside host-expanded group
scales; on-chip dequant is a single chunked tensor_tensor multiply (DVE
for QKV at startup, GpSimd for the wo panels so they overlap the last
batch's attention). The causal mask is a 0/1 multiply after exp, and
softmax normalization divides after the PV matmul (linearity), with the
1/z partition-broadcast on GpSimd.
"""

import math
from dataclasses import dataclass

import numpy as np

import concourse.bass as bass
import concourse.tile as tile
from concourse.masks import make_identity
from concourse import bacc, mybir

BF = mybir.dt.bfloat16
F32 = mybir.dt.float32
I8 = mybir.dt.int8
AOP = mybir.AluOpType
AF = mybir.ActivationFunctionType


@dataclass
class Cfg:
    B: int = 4
    S: int = 1024
    D: int = 4096
    NCORES: int = 8
    SCH: int = 512   # kept for test.py compat (unused)
    QCH: int = 512   # attention q-chunk

    @property
    def T(self):
        return self.B * self.S

    @property
    def H(self):
        return self.D // 128  # total heads (head_dim 128)

    @property
    def H_LOC(self):
        return self.H // self.NCORES

    @property
    def C_SHARD(self):
        return self.H_LOC * 128  # local channels

    @property
    def TPC(self):
        return self.T // self.NCORES  # tokens per core (output slice)

    @property
    def NGP(self):
        return self.D // 128  # contraction k-tiles per row


def build_program(cfg: Cfg):
    """Build the per-core Bass program. Returns compiled nc."""
    c = cfg
    assert c.QCH == 512 and c.S == 1024 and c.NCORES == 8

    import concourse.tile_utils as tile_utils
    tile_utils.max_sbuf_usage = 208 * 1024

    nc = bacc.Bacc("TRN2", target_bir_lowering=False, debug=False,
                   num_devices=c.NCORES)

    OSH = c.C_SHARD          # qkv weight shard out-channels per core (512)
    NGP = c.NGP              # 32
    NTIL = c.T // 128        # 32 global token tiles
    TPB = c.S // 128         # 8 tiles per batch

    # ---- external I/O ----
    # x retiled: [p=i%128, tile, g=i//128, t']
    x_d = nc.dram_tensor("x", [128, NTIL, NGP, 128], BF, kind="ExternalInput")
    # unpacked int4 values, transposed: wt[p=i%128, g=i//128, o]
    w_q = nc.dram_tensor("wq_w", [128, NGP, OSH], I8, kind="ExternalInput")
    s_q = nc.dram_tensor("wq_s", [128, NGP, OSH], BF, kind="ExternalInput")
    w_k = nc.dram_tensor("wk_w", [128, NGP, OSH], I8, kind="ExternalInput")
    s_k = nc.dram_tensor("wk_s", [128, NGP, OSH], BF, kind="ExternalInput")
    w_v = nc.dram_tensor("wv_w", [128, NGP, OSH], I8, kind="ExternalInput")
    s_v = nc.dram_tensor("wv_s", [128, NGP, OSH], BF, kind="ExternalInput")
    # wo panel-major: [p, oc, g, o']
    w_o = nc.dram_tensor("wo_w", [128, c.D // 512, NGP, 512], I8,
                         kind="ExternalInput")
    s_o = nc.dram_tensor("wo_s", [128, c.D // 512, NGP, 512], BF,
                         kind="ExternalInput")
    # rope tables, compact: [p=s%128, ssub=s//128, d]
    cosc_d = nc.dram_tensor("cosc", [128, TPB, 128], BF, kind="ExternalInput")
    sinc_d = nc.dram_tensor("sinc", [128, TPB, 128], BF, kind="ExternalInput")
    maskd_d = nc.dram_tensor("maskd", [128, 128], BF, kind="ExternalInput")
    out_d = nc.dram_tensor("out", [c.TPC, c.D], BF, kind="ExternalOutput")

    # per-batch collective bounce buffers; slot j = within-batch token tile j
    a2a_in = [nc.dram_tensor(f"a2a_in{b}", [c.NCORES, c.C_SHARD, 128], BF)
              for b in range(c.B)]
    a2a_out = [nc.dram_tensor(f"a2a_out{b}", [c.NCORES, c.C_SHARD, 128], BF)
               for b in range(c.B)]

    inv_sqrt_d = 1.0 / math.sqrt(128.0)

    def dequant_t(pool, wt, bt_ap, sc_ap, ngp, osz, chunks=4, eng=None):
        """Dequantize unpacked int4 values into transposed wt [128, ngp, osz].

        bt_ap: DRAM [128, ngp, osz] int8 values; sc_ap: DRAM [128, ngp, osz]
        host-expanded scales. Works in double-buffered [128, ngp/chunks, osz]
        chunk tiles so DMAs pipeline and consumers can start early."""
        if eng is None:
            eng = nc.vector
        gch = ngp // chunks
        for i in range(chunks):
            g0 = i * gch
            nq = pool.tile([128, gch, osz], I8, tag="dq_nb", bufs=2)
            sc = pool.tile([128, gch, osz], BF, tag="dq_sc", bufs=2)
            nc.sync.dma_start(nq[:], bt_ap[:, g0:g0 + gch, :])
            nc.sync.dma_start(sc[:], sc_ap[:, g0:g0 + gch, :])
            eng.tensor_tensor(
                out=wt[:, g0:g0 + gch, :], in0=nq[:], in1=sc[:],
                op=AOP.mult)

    with tile.TileContext(nc) as tc:
        with tc.tile_pool(name="const", bufs=1) as const, \
             tc.tile_pool(name="sbuf", bufs=2) as sbuf:
            # constants
            cosc = const.tile([128, TPB, 128], BF)
            nc.sync.dma_start(cosc[:], cosc_d[:])
            sinc = const.tile([128, TPB, 128], BF)
            nc.sync.dma_start(sinc[:], sinc_d[:])
            maskd = const.tile([128, 128], BF)
            nc.sync.dma_start(maskd[:], maskd_d[:])
            ones_col = const.tile([128, 1], BF)
            nc.vector.memset(ones_col[:], 1.0)
            ones_row = const.tile([1, 128], BF)
            nc.vector.memset(ones_row[:], 1.0)
            ident = const.tile([128, 128], BF)
            make_identity(nc, ident)

            # ============ phase 1: QKV + attention ============
            with tc.tile_pool(name="wt", bufs=1) as wtp:
                wt_q = wtp.tile([128, NGP, OSH], BF, tag="wt_q")
                wt_k = wtp.tile([128, NGP, OSH], BF, tag="wt_k")
                wt_v = wtp.tile([128, NGP, OSH], BF, tag="wt_v")
                with tc.tile_pool(name="dqp", bufs=1) as dqp, \
                     tc.tile_pool(name="xt", bufs=1) as xtp, \
                     tc.tile_pool(name="kqv", bufs=2) as kqvp, \
                     tc.tile_pool(name="pt", bufs=4) as ptp, \
                     tc.tile_pool(name="ppsum", bufs=2, space="PSUM") as ppsum, \
                     tc.tile_pool(name="spsum", bufs=2, space="PSUM") as spsum, \
                     tc.tile_pool(name="zpsum", bufs=1, space="PSUM") as zpsum, \
                     tc.tile_pool(name="apsum", bufs=1, space="PSUM") as apsum, \
                     tc.tile_pool(name="tpsum", bufs=2, space="PSUM") as tpsum:

                    def proj_one(mat, wt_m, xt_ts, ts, kt_b, qt_b, v_b):
                        st0 = ts * 128
                        ps = ppsum.tile([128, OSH], F32, tag="proj")
                        for gp in range(NGP):
                            nc.tensor.matmul(
                                ps[:],
                                lhsT=xt_ts[:, gp, :],
                                rhs=wt_m[:, gp, :],
                                start=(gp == 0),
                                stop=(gp == NGP - 1))
                        if mat == "v":
                            nc.scalar.copy(out=v_b[:, ts, :], in_=ps[:])
                            return
                        # single PSUM read, then rope from SBUF bf16
                        psc = sbuf.tile([128, c.C_SHARD], BF,
                                        tag="psc", bufs=2)
                        nc.scalar.copy(out=psc[:], in_=ps[:])
                        # rope: roped = psc*cos + swaphalf(psc)*sin(+/-)
                        roped = sbuf.tile([128, c.C_SHARD], BF,
                                          tag="roped", bufs=2)
                        tmp = sbuf.tile([128, c.C_SHARD], BF,
                                        tag="ropetmp", bufs=2)
                        p3 = psc[:].rearrange("p (h d) -> p h d", d=128)
                        t3 = tmp[:].rearrange("p (h d) -> p h d", d=128)
                        r3 = roped[:].rearrange("p (h d) -> p h d", d=128)
                        c3 = cosc[:, ts, :][:, None, :].to_broadcast(
                            [128, c.H_LOC, 128])
                        s3 = sinc[:, ts, :][:, None, :].to_broadcast(
                            [128, c.H_LOC, 128])
                        nc.vector.tensor_tensor(
                            out=t3[:, :, 0:64], in0=p3[:, :, 64:128],
                            in1=s3[:, :, 0:64], op=AOP.mult)
                        nc.vector.tensor_tensor(
                            out=t3[:, :, 64:128], in0=p3[:, :, 0:64],
                            in1=s3[:, :, 64:128], op=AOP.mult)
                        nc.vector.tensor_tensor(
                            out=r3[:], in0=p3[:], in1=c3, op=AOP.mult)
                        nc.vector.tensor_tensor(
                            out=roped[:], in0=roped[:], in1=tmp[:],
                            op=AOP.add)
                        dst = qt_b if mat == "q" else kt_b
                        for h in range(c.H_LOC):
                            tp = tpsum.tile([128, 128], BF, tag="tp",
                                            bufs=2)
                            nc.tensor.transpose(
                                tp[:], roped[:, h * 128:(h + 1) * 128],
                                ident[:])
                            if h % 2 == 0:
                                nc.scalar.copy(
                                    out=dst[:, h, st0:st0 + 128], in_=tp[:])
                            else:
                                nc.vector.tensor_copy(
                                    out=dst[:, h, st0:st0 + 128], in_=tp[:])

                    dq_srcs = {"q": (wt_q, w_q, s_q), "k": (wt_k, w_k, s_k),
                               "v": (wt_v, w_v, s_v)}
                    for b in range(c.B):
                        # per-batch K/Q transposed and V natural
                        kt_b = kqvp.tile([128, c.H_LOC, c.S], BF, tag="kt_b")
                        qt_b = kqvp.tile([128, c.H_LOC, c.S], BF, tag="qt_b")
                        v_b = kqvp.tile([128, TPB, c.C_SHARD], BF, tag="v_b",
                                        bufs=1)
                        if b == 0:
                            # mat-outer, dequant interleaved: q projections
                            # start right after wq's dequant while k/v still
                            # dequantize (x tiles re-loaded per mat)
                            for mat, wt_m in (("q", wt_q), ("k", wt_k),
                                              ("v", wt_v)):
                                _, w_m, s_m = dq_srcs[mat]
                                # wq on DVE (finest chunks, PE starts asap);
                                # wk/wv on the idle GpSimd so DVE is free for
                                # batch 0's rope (no collectives pending yet,
                                # so no gpsimd queue entanglement here)
                                dequant_t(dqp, wt_m, w_m.ap(), s_m.ap(),
                                          NGP, OSH,
                                          chunks=16 if mat == "q" else 8,
                                          eng=(nc.vector if mat == "q"
                                               else nc.gpsimd))
                                for ts in range(TPB):
                                    xt_ts = xtp.tile([128, NGP, 128], BF,
                                                     tag="xt", bufs=2)
                                    nc.sync.dma_start(xt_ts[:],
                                                      x_d.ap()[:, ts])
                                    proj_one(mat, wt_m, xt_ts, ts,
                                             kt_b, qt_b, v_b)
                        else:
                            for ts in range(TPB):
                                tt = b * TPB + ts
                                xt_ts = xtp.tile([128, NGP, 128], BF,
                                                 tag="xt", bufs=2)
                                nc.sync.dma_start(xt_ts[:], x_d.ap()[:, tt])
                                for mat, wt_m in (("q", wt_q), ("k", wt_k),
                                                  ("v", wt_v)):
                                    proj_one(mat, wt_m, xt_ts, ts,
                                             kt_b, qt_b, v_b)

                        # ---- attention for batch b ----
                        for h in range(c.H_LOC):
                            for qc in range(c.S // c.QCH):
                                q0 = qc * c.QCH
                                kmax = (q0 + c.QCH) // 128
                                at = apsum.tile([128, c.QCH], F32, tag="at")
                                zp = zpsum.tile([1, c.QCH], F32, tag="z")
                                psum_tree = sbuf.tile([128, c.QCH], BF,
                                                      tag="ptree", bufs=2)
                                for ki in range(kmax):
                                    off = max(0, 128 * ki - q0)
                                    stp = spsum.tile([128, c.QCH], F32,
                                                     tag="sc")
                                    nc.tensor.matmul(
                                        stp[:, off:],
                                        lhsT=kt_b[:, h,
                                                  ki * 128:(ki + 1) * 128],
                                        rhs=qt_b[:, h, q0 + off:q0 + c.QCH],
                                        start=True, stop=True)
                                    pt = ptp.tile([128, c.QCH], BF, tag="pt")
                                    nc.scalar.activation(
                                        out=pt[:, off:], in_=stp[:, off:],
                                        func=AF.Exp, scale=inv_sqrt_d)
                                    if 128 * ki >= q0:
                                        # zero the upper triangle of the
                                        # diagonal block (causal mask)
                                        nc.vector.tensor_tensor(
                                            out=pt[:, off:off + 128],
                                            in0=pt[:, off:off + 128],
                                            in1=maskd[:], op=AOP.mult)
                                    if ki == 0:
                                        nc.vector.tensor_copy(
                                            out=psum_tree[:], in_=pt[:])
                                    else:
                                        nc.vector.tensor_tensor(
                                            out=psum_tree[:, off:],
                                            in0=psum_tree[:, off:],
                                            in1=pt[:, off:], op=AOP.add)
                                    nc.tensor.matmul(
                                        at[:, off:],
                                        lhsT=v_b[:, ki,
                                                 h * 128:(h + 1) * 128],
                                        rhs=pt[:, off:],
                                        start=(ki == 0),
                                        stop=(ki == kmax - 1))
                                nc.tensor.matmul(
                                    zp[:], lhsT=ones_col[:], rhs=psum_tree[:],
                                    start=True, stop=True)
                                rz = sbuf.tile([1, c.QCH], F32, tag="rz")
                                nc.vector.reciprocal_approx_fast(rz[:], zp[:])
                                bzs = sbuf.tile([128, c.QCH], F32, tag="bzs")
                                nc.gpsimd.partition_broadcast(bzs[:], rz[:])
                                ao = sbuf.tile([128, c.QCH], BF, tag="ao")
                                nc.vector.tensor_tensor(
                                    out=ao[:], in0=at[:], in1=bzs[:],
                                    op=AOP.mult)
                                for j in range(c.QCH // 128):
                                    slot = qc * (c.QCH // 128) + j
                                    nc.sync.dma_start(
                                        out=a2a_in[b][slot][
                                            h * 128:(h + 1) * 128, :],
                                        in_=ao[:, j * 128:(j + 1) * 128])

                        # per-batch collective, overlaps next batch's compute
                        nc.gpsimd.collective_compute(
                            "AllToAll", AOP.bypass,
                            replica_groups=[list(range(c.NCORES))],
                            ins=[a2a_in[b].ap().opt()],
                            outs=[a2a_out[b].ap().opt()],
                        )

            # ============ phase 2: output projection (token-sharded) ============
            with tc.tile_pool(name="gath", bufs=1) as gathp, \
                 tc.tile_pool(name="p2", bufs=1) as p2p, \
                 tc.tile_pool(name="wpsum", bufs=2, space="PSUM") as wpsum:
                gaths = []
                for b in range(c.B):
                    g = gathp.tile([128, NGP, 128], BF, tag=f"gath{b}")
                    nc.sync.dma_start(
                        g[:],
                        a2a_out[b].ap().rearrange(
                            "r (g p) t -> p (r g) t", p=128))
                    gaths.append(g)
                def wo_gemm(oc, b, panel):
                    ops = wpsum.tile([128, 512], F32, tag="wo")
                    for ct in range(NGP):
                        nc.tensor.matmul(
                            ops[:], lhsT=gaths[b][:, ct, :],
                            rhs=panel[:, ct, :],
                            start=(ct == 0), stop=(ct == NGP - 1))
                    osb = sbuf.tile([128, 512], BF, tag="osb", bufs=2)
                    nc.scalar.copy(out=osb[:], in_=ops[:])
                    nc.sync.dma_start(
                        out=out_d[b * 128:(b + 1) * 128,
                                  oc * 512:(oc + 1) * 512],
                        in_=osb[:])

                # The last batch's GEMMs wait on its collective, which is
                # delayed by cross-core skew; push the first panels' b3 GEMMs
                # to the very end (cheap re-dequant) so nothing stalls on it.
                DEFER = 2
                NOC = c.D // 512
                tail = []
                for oc in range(NOC + DEFER):
                    src_oc = oc if oc < NOC else oc - NOC
                    panel = p2p.tile([128, NGP, 512], BF, tag="wop", bufs=2)
                    dequant_t(p2p, panel, w_o.ap()[:, src_oc],
                              s_o.ap()[:, src_oc], NGP, 512,
                              chunks=8 if oc == 0 else 4)
                    if oc < NOC:
                        nb = c.B - 1 if oc < DEFER else c.B
                        for b in range(nb):
                            wo_gemm(oc, b, panel)
                        tail.append(None)
                    else:
                        wo_gemm(src_oc, c.B - 1, panel)

    nc.compile()
    return nc


# ---------------- host-side input prep ----------------

def prep_core_inputs(cfg: Cfg, x, cos_half, sin_half, mask,
                     wq_w, wq_s, wk_w, wk_s, wv_w, wv_s, wo_w, wo_s):
    """Build in_maps (list of dicts, one per core) from full inputs."""
    import ml_dtypes
    c = cfg
    bf16 = ml_dtypes.bfloat16
    HD2 = 64
    NGP = c.NGP
    OSH = c.C_SHARD
    TPB = c.S // 128

    # x retiled: [p=i%128, tile, g=i//128, t']
    x5 = np.ascontiguousarray(
        np.asarray(x).reshape(c.T // 128, 128, NGP, 128).transpose(3, 0, 2, 1)
    ).astype(bf16, copy=False)

    # rope tables [128, TPB, 128], compact (broadcast over heads on-chip)
    ch = np.asarray(cos_half, np.float32)  # [S, 64]
    sh = np.asarray(sin_half, np.float32)
    cos = np.concatenate([ch, ch], axis=1).astype(bf16).astype(np.float32)
    sin = np.concatenate([sh, sh], axis=1).astype(bf16).astype(np.float32)
    sins = sin.copy()
    sins[:, :HD2] = -sin[:, :HD2]
    cosc = np.ascontiguousarray(
        cos.reshape(TPB, 128, 128).transpose(1, 0, 2)).astype(bf16)
    sinc = np.ascontiguousarray(
        sins.reshape(TPB, 128, 128).transpose(1, 0, 2)).astype(bf16)

    # diagonal 0/1 mask block: maskd[k, q] = 1 where mask[q, k] == 0
    m = np.asarray(mask, np.float32)[:128, :128]
    maskd = (m.T == 0.0).astype(np.float32).astype(bf16)

    def pack_w(pw, o_n, panel=None):
        """packed [o_n*NGP, 64] -> unpacked int4 values [128, NGP, o_n]
        with w[p, g, o] = W_q[o, 128*g + p] (or panel-major 4D)."""
        a = np.asarray(pw).reshape(o_n, NGP, 64)
        msb = (a >> 4).astype(np.int8)                    # i = 128g + f
        lsb = (((a & 15) ^ 8) - 8).astype(np.int8)        # i = 128g + 64 + f
        full = np.concatenate(
            [msb.transpose(2, 1, 0), lsb.transpose(2, 1, 0)], axis=0)
        if panel is None:
            return np.ascontiguousarray(full)             # [128, NGP, o_n]
        full = full.reshape(128, NGP, panel, o_n // panel)
        return np.ascontiguousarray(full.transpose(0, 2, 1, 3))

    def pack_s(ps, o_n, panel=None):
        """scales [o_n*2*NGP, 1] -> host-expanded [128, NGP, o_n]
        (rows 0:64 msb scale, 64:128 lsb scale), or panel-major 4D."""
        a = np.asarray(ps).astype(np.float32).reshape(o_n, NGP, 2)
        two = a.transpose(2, 1, 0)  # [2, NGP, o_n]
        full = np.concatenate([
            np.broadcast_to(two[0:1], (64, NGP, o_n)),
            np.broadcast_to(two[1:2], (64, NGP, o_n))], axis=0)
        if panel is None:
            return np.ascontiguousarray(full).astype(bf16)
        full = full.reshape(128, NGP, panel, o_n // panel)
        return np.ascontiguousarray(full.transpose(0, 2, 1, 3)).astype(bf16)

    wo_bt = pack_w(wo_w, c.D, panel=c.D // 512)
    wo_sc = pack_s(wo_s, c.D, panel=c.D // 512)

    in_maps = []
    for core in range(c.NCORES):
        r0 = core * OSH * NGP
        g0 = core * OSH * 2 * NGP
        in_maps.append({
            "x": x5,
            "wq_w": pack_w(np.asarray(wq_w)[r0:r0 + OSH * NGP], OSH),
            "wq_s": pack_s(np.asarray(wq_s)[g0:g0 + OSH * 2 * NGP], OSH),
            "wk_w": pack_w(np.asarray(wk_w)[r0:r0 + OSH * NGP], OSH),
            "wk_s": pack_s(np.asarray(wk_s)[g0:g0 + OSH * 2 * NGP], OSH),
            "wv_w": pack_w(np.asarray(wv_w)[r0:r0 + OSH * NGP], OSH),
            "wv_s": pack_s(np.asarray(wv_s)[g0:g0 + OSH * 2 * NGP], OSH),
            "wo_w": wo_bt,
            "wo_s": wo_sc,
            "cosc": cosc,
            "sinc": sinc,
            "maskd": maskd,
        })
    return in_maps


def unshard_output(cfg: Cfg, results):
    """results: list per core of {"out": [TPC, D]}. Returns [B, S, D].

    Core j's output rows b*128:(b+1)*128 hold global token tile 8*b + j."""
    c = cfg
    TPB = c.S // 128
    full = np.empty((c.B * TPB, 128, c.D),
                    dtype=np.asarray(results[0]["out"]).dtype)
    for j in range(c.NCORES):
        o = np.asarray(results[j]["out"]).reshape(c.B, 128, c.D)
        for b in range(c.B):
            full[TPB * b + j] = o[b]
    return full.reshape(c.B, c.S, c.D)


# ======================================================================
# Self-contained kernel entry point.
# ======================================================================

_CACHE = {}


def _get_program(cfg):
    key = (cfg.B, cfg.S, cfg.D, cfg.NCORES, cfg.SCH, cfg.QCH)
    if key not in _CACHE:
        _CACHE[key] = build_program(cfg)
    return _CACHE[key]


def kernel(x, start_pos=0, cos_half=None, sin_half=None, mask=None,
           wq_w=None, wq_s=None, wk_w=None, wk_s=None,
           wv_w=None, wv_s=None, wo_w=None, wo_s=None,
           cache_k_w=None, cache_k_s=None, cache_v_w=None, cache_v_s=None,
           **_unused):
    from concourse.bass_utils import run_bass_kernel_spmd

    assert int(start_pos) == 0, "kernel specialised for start_pos == 0"
    x = np.asarray(x)
    B, S, D = x.shape
    cfg = Cfg(B=B, S=S, D=D, NCORES=8, SCH=512, QCH=512)
    # start_pos==0 with S==MAX_S, B==MAX_B: the quantized KV cache is fully
    # overwritten before use, so cache_* inputs cannot affect the output.
    in_maps = prep_core_inputs(cfg, x, cos_half, sin_half, mask,
                               wq_w, wq_s, wk_w, wk_s, wv_w, wv_s,
                               wo_w, wo_s)
    nc = _get_program(cfg)
    res = run_bass_kernel_spmd(nc, in_maps, core_ids=list(range(cfg.NCORES)))
    out = unshard_output(cfg, res.results)
    import ml_dtypes
    return out.astype(ml_dtypes.bfloat16, copy=False)


# revision 48
# speedup vs baseline: 1.0289x; 1.0085x over previous
"""Trainium2 (Bass/Tile) kernel for quantized multi-head attention.

Distributed across 8 NeuronCores: tensor-parallel over heads for the
QKV projections + RoPE + causal attention, per-batch AllToAll
collectives (overlapped with later batches' compute), then a
token-parallel output projection over interleaved 128-token tiles.

The Q4_0 weights ship host-UNPACKED (int4 values widened to int8, laid
out transposed [in%128, in//128, out] so dequant lands directly in the
matmul-rhs layout with no PE transposes) alongside host-expanded group
scales; on-chip dequant is a single chunked tensor_tensor multiply (DVE
for QKV at startup, GpSimd for the wo panels so they overlap the last
batch's attention). The causal mask is a 0/1 multiply after exp, and
softmax normalization divides after the PV matmul (linearity), with the
1/z partition-broadcast on GpSimd.
"""

import math
from dataclasses import dataclass

import numpy as np

import concourse.bass as bass
import concourse.tile as tile
from concourse.masks import make_identity
from concourse import bacc, mybir

BF = mybir.dt.bfloat16
F32 = mybir.dt.float32
I8 = mybir.dt.int8
AOP = mybir.AluOpType
AF = mybir.ActivationFunctionType


@dataclass
class Cfg:
    B: int = 4
    S: int = 1024
    D: int = 4096
    NCORES: int = 8
    SCH: int = 512   # kept for test.py compat (unused)
    QCH: int = 512   # attention q-chunk

    @property
    def T(self):
        return self.B * self.S

    @property
    def H(self):
        return self.D // 128  # total heads (head_dim 128)

    @property
    def H_LOC(self):
        return self.H // self.NCORES

    @property
    def C_SHARD(self):
        return self.H_LOC * 128  # local channels

    @property
    def TPC(self):
        return self.T // self.NCORES  # tokens per core (output slice)

    @property
    def NGP(self):
        return self.D // 128  # contraction k-tiles per row


def build_program(cfg: Cfg):
    """Build the per-core Bass program. Returns compiled nc."""
    c = cfg
    assert c.QCH == 512 and c.S == 1024 and c.NCORES == 8

    import concourse.tile_utils as tile_utils
    tile_utils.max_sbuf_usage = 208 * 1024

    nc = bacc.Bacc("TRN2", target_bir_lowering=False, debug=False,
                   num_devices=c.NCORES)

    OSH = c.C_SHARD          # qkv weight shard out-channels per core (512)
    NGP = c.NGP              # 32
    NTIL = c.T // 128        # 32 global token tiles
    TPB = c.S // 128         # 8 tiles per batch

    # ---- external I/O ----
    # x retiled: [p=i%128, tile, g=i//128, t']
    x_d = nc.dram_tensor("x", [128, NTIL, NGP, 128], BF, kind="ExternalInput")
    # unpacked int4 values, transposed: wt[p=i%128, g=i//128, o]
    w_q = nc.dram_tensor("wq_w", [128, NGP, OSH], I8, kind="ExternalInput")
    s_q = nc.dram_tensor("wq_s", [128, NGP, OSH], BF, kind="ExternalInput")
    w_k = nc.dram_tensor("wk_w", [128, NGP, OSH], I8, kind="ExternalInput")
    s_k = nc.dram_tensor("wk_s", [128, NGP, OSH], BF, kind="ExternalInput")
    w_v = nc.dram_tensor("wv_w", [128, NGP, OSH], I8, kind="ExternalInput")
    s_v = nc.dram_tensor("wv_s", [128, NGP, OSH], BF, kind="ExternalInput")
    # wo panel-major: [p, oc, g, o']
    w_o = nc.dram_tensor("wo_w", [128, c.D // 512, NGP, 512], I8,
                         kind="ExternalInput")
    s_o = nc.dram_tensor("wo_s", [128, c.D // 512, NGP, 512], BF,
                         kind="ExternalInput")
    # rope tables, compact: [p=s%128, ssub=s//128, d]
    cosc_d = nc.dram_tensor("cosc", [128, TPB, 128], BF, kind="ExternalInput")
    sinc_d = nc.dram_tensor("sinc", [128, TPB, 128], BF, kind="ExternalInput")
    maskd_d = nc.dram_tensor("maskd", [128, 128], BF, kind="ExternalInput")
    out_d = nc.dram_tensor("out", [c.TPC, c.D], BF, kind="ExternalOutput")

    # per-batch collective bounce buffers; slot j = within-batch token tile j
    a2a_in = [nc.dram_tensor(f"a2a_in{b}", [c.NCORES, c.C_SHARD, 128], BF)
              for b in range(c.B)]
    a2a_out = [nc.dram_tensor(f"a2a_out{b}", [c.NCORES, c.C_SHARD, 128], BF)
               for b in range(c.B)]

    inv_sqrt_d = 1.0 / math.sqrt(128.0)

    def dequant_t(pool, wt, bt_ap, sc_ap, ngp, osz, chunks=4, eng=None):
        """Dequantize unpacked int4 values into transposed wt [128, ngp, osz].

        bt_ap: DRAM [128, ngp, osz] int8 values; sc_ap: DRAM [128, ngp, osz]
        host-expanded scales. Works in double-buffered [128, ngp/chunks, osz]
        chunk tiles so DMAs pipeline and consumers can start early."""
        if eng is None:
            eng = nc.vector
        gch = ngp // chunks
        for i in range(chunks):
            g0 = i * gch
            nq = pool.tile([128, gch, osz], I8, tag="dq_nb", bufs=2)
            sc = pool.tile([128, gch, osz], BF, tag="dq_sc", bufs=2)
            nc.sync.dma_start(nq[:], bt_ap[:, g0:g0 + gch, :])
            nc.sync.dma_start(sc[:], sc_ap[:, g0:g0 + gch, :])
            eng.tensor_tensor(
                out=wt[:, g0:g0 + gch, :], in0=nq[:], in1=sc[:],
                op=AOP.mult)

    with tile.TileContext(nc) as tc:
        with tc.tile_pool(name="const", bufs=1) as const, \
             tc.tile_pool(name="sbuf", bufs=2) as sbuf:
            # constants
            cosc = const.tile([128, TPB, 128], BF)
            nc.sync.dma_start(cosc[:], cosc_d[:])
            sinc = const.tile([128, TPB, 128], BF)
            nc.sync.dma_start(sinc[:], sinc_d[:])
            maskd = const.tile([128, 128], BF)
            nc.sync.dma_start(maskd[:], maskd_d[:])
            ones_col = const.tile([128, 1], BF)
            nc.vector.memset(ones_col[:], 1.0)
            ones_row = const.tile([1, 128], BF)
            nc.vector.memset(ones_row[:], 1.0)
            ident = const.tile([128, 128], BF)
            make_identity(nc, ident)

            # ============ phase 1: QKV + attention ============
            with tc.tile_pool(name="wt", bufs=1) as wtp:
                wt_q = wtp.tile([128, NGP, OSH], BF, tag="wt_q")
                wt_k = wtp.tile([128, NGP, OSH], BF, tag="wt_k")
                wt_v = wtp.tile([128, NGP, OSH], BF, tag="wt_v")
                with tc.tile_pool(name="dqp", bufs=1) as dqp, \
                     tc.tile_pool(name="xt", bufs=1) as xtp, \
                     tc.tile_pool(name="kqv", bufs=2) as kqvp, \
                     tc.tile_pool(name="pt", bufs=4) as ptp, \
                     tc.tile_pool(name="ppsum", bufs=2, space="PSUM") as ppsum, \
                     tc.tile_pool(name="spsum", bufs=2, space="PSUM") as spsum, \
                     tc.tile_pool(name="zpsum", bufs=1, space="PSUM") as zpsum, \
                     tc.tile_pool(name="apsum", bufs=1, space="PSUM") as apsum, \
                     tc.tile_pool(name="tpsum", bufs=2, space="PSUM") as tpsum:

                    def proj_one(mat, wt_m, xt_ts, ts, kt_b, qt_b, v_b):
                        st0 = ts * 128
                        ps = ppsum.tile([128, OSH], F32, tag="proj")
                        for gp in range(NGP):
                            nc.tensor.matmul(
                                ps[:],
                                lhsT=xt_ts[:, gp, :],
                                rhs=wt_m[:, gp, :],
                                start=(gp == 0),
                                stop=(gp == NGP - 1))
                        if mat == "v":
                            nc.scalar.copy(out=v_b[:, ts, :], in_=ps[:])
                            return
                        # single PSUM read, then rope from SBUF bf16
                        psc = sbuf.tile([128, c.C_SHARD], BF,
                                        tag="psc", bufs=2)
                        nc.scalar.copy(out=psc[:], in_=ps[:])
                        # rope: roped = psc*cos + swaphalf(psc)*sin(+/-)
                        roped = sbuf.tile([128, c.C_SHARD], BF,
                                          tag="roped", bufs=2)
                        tmp = sbuf.tile([128, c.C_SHARD], BF,
                                        tag="ropetmp", bufs=2)
                        p3 = psc[:].rearrange("p (h d) -> p h d", d=128)
                        t3 = tmp[:].rearrange("p (h d) -> p h d", d=128)
                        r3 = roped[:].rearrange("p (h d) -> p h d", d=128)
                        c3 = cosc[:, ts, :][:, None, :].to_broadcast(
                            [128, c.H_LOC, 128])
                        s3 = sinc[:, ts, :][:, None, :].to_broadcast(
                            [128, c.H_LOC, 128])
                        nc.vector.tensor_tensor(
                            out=t3[:, :, 0:64], in0=p3[:, :, 64:128],
                            in1=s3[:, :, 0:64], op=AOP.mult)
                        nc.vector.tensor_tensor(
                            out=t3[:, :, 64:128], in0=p3[:, :, 0:64],
                            in1=s3[:, :, 64:128], op=AOP.mult)
                        nc.vector.tensor_tensor(
                            out=r3[:], in0=p3[:], in1=c3, op=AOP.mult)
                        nc.vector.tensor_tensor(
                            out=roped[:], in0=roped[:], in1=tmp[:],
                            op=AOP.add)
                        dst = qt_b if mat == "q" else kt_b
                        for h in range(c.H_LOC):
                            tp = tpsum.tile([128, 128], BF, tag="tp",
                                            bufs=2)
                            nc.tensor.transpose(
                                tp[:], roped[:, h * 128:(h + 1) * 128],
                                ident[:])
                            if h % 2 == 0:
                                nc.scalar.copy(
                                    out=dst[:, h, st0:st0 + 128], in_=tp[:])
                            else:
                                nc.vector.tensor_copy(
                                    out=dst[:, h, st0:st0 + 128], in_=tp[:])

                    dq_srcs = {"q": (wt_q, w_q, s_q), "k": (wt_k, w_k, s_k),
                               "v": (wt_v, w_v, s_v)}
                    for b in range(c.B):
                        # per-batch K/Q transposed and V natural
                        kt_b = kqvp.tile([128, c.H_LOC, c.S], BF, tag="kt_b")
                        qt_b = kqvp.tile([128, c.H_LOC, c.S], BF, tag="qt_b")
                        v_b = kqvp.tile([128, TPB, c.C_SHARD], BF, tag="v_b",
                                        bufs=1)
                        if b == 0:
                            # mat-outer, dequant pipelined one matrix AHEAD:
                            # each mat's dequant DVE chunks are emitted
                            # before the PREVIOUS mat's rope work so they
                            # never trail the PE (x tiles re-loaded per mat)
                            mats = (("q", wt_q), ("k", wt_k), ("v", wt_v))
                            for mi, (mat, wt_m) in enumerate(mats):
                                if mi == 0:
                                    for pmat, pwt in mats[:2]:
                                        _, w_m, s_m = dq_srcs[pmat]
                                        dequant_t(dqp, pwt, w_m.ap(),
                                                  s_m.ap(), NGP, OSH,
                                                  chunks=8)
                                elif mi == 1:
                                    _, w_m, s_m = dq_srcs["v"]
                                    dequant_t(dqp, wt_v, w_m.ap(),
                                              s_m.ap(), NGP, OSH, chunks=8)
                                for ts in range(TPB):
                                    xt_ts = xtp.tile([128, NGP, 128], BF,
                                                     tag="xt", bufs=2)
                                    nc.sync.dma_start(xt_ts[:],
                                                      x_d.ap()[:, ts])
                                    proj_one(mat, wt_m, xt_ts, ts,
                                             kt_b, qt_b, v_b)
                        else:
                            for ts in range(TPB):
                                tt = b * TPB + ts
                                xt_ts = xtp.tile([128, NGP, 128], BF,
                                                 tag="xt", bufs=2)
                                nc.sync.dma_start(xt_ts[:], x_d.ap()[:, tt])
                                for mat, wt_m in (("q", wt_q), ("k", wt_k),
                                                  ("v", wt_v)):
                                    proj_one(mat, wt_m, xt_ts, ts,
                                             kt_b, qt_b, v_b)

                        # ---- attention for batch b ----
                        for h in range(c.H_LOC):
                            for qc in range(c.S // c.QCH):
                                q0 = qc * c.QCH
                                kmax = (q0 + c.QCH) // 128
                                at = apsum.tile([128, c.QCH], F32, tag="at")
                                zp = zpsum.tile([1, c.QCH], F32, tag="z")
                                psum_tree = sbuf.tile([128, c.QCH], BF,
                                                      tag="ptree", bufs=2)
                                for ki in range(kmax):
                                    off = max(0, 128 * ki - q0)
                                    stp = spsum.tile([128, c.QCH], F32,
                                                     tag="sc")
                                    nc.tensor.matmul(
                                        stp[:, off:],
                                        lhsT=kt_b[:, h,
                                                  ki * 128:(ki + 1) * 128],
                                        rhs=qt_b[:, h, q0 + off:q0 + c.QCH],
                                        start=True, stop=True)
                                    pt = ptp.tile([128, c.QCH], BF, tag="pt")
                                    nc.scalar.activation(
                                        out=pt[:, off:], in_=stp[:, off:],
                                        func=AF.Exp, scale=inv_sqrt_d)
                                    if 128 * ki >= q0:
                                        # zero the upper triangle of the
                                        # diagonal block (causal mask)
                                        nc.vector.tensor_tensor(
                                            out=pt[:, off:off + 128],
                                            in0=pt[:, off:off + 128],
                                            in1=maskd[:], op=AOP.mult)
                                    if ki == 0:
                                        nc.vector.tensor_copy(
                                            out=psum_tree[:], in_=pt[:])
                                    else:
                                        nc.vector.tensor_tensor(
                                            out=psum_tree[:, off:],
                                            in0=psum_tree[:, off:],
                                            in1=pt[:, off:], op=AOP.add)
                                    nc.tensor.matmul(
                                        at[:, off:],
                                        lhsT=v_b[:, ki,
                                                 h * 128:(h + 1) * 128],
                                        rhs=pt[:, off:],
                                        start=(ki == 0),
                                        stop=(ki == kmax - 1))
                                nc.tensor.matmul(
                                    zp[:], lhsT=ones_col[:], rhs=psum_tree[:],
                                    start=True, stop=True)
                                rz = sbuf.tile([1, c.QCH], F32, tag="rz")
                                nc.vector.reciprocal_approx_fast(rz[:], zp[:])
                                bzs = sbuf.tile([128, c.QCH], F32, tag="bzs")
                                nc.gpsimd.partition_broadcast(bzs[:], rz[:])
                                ao = sbuf.tile([128, c.QCH], BF, tag="ao")
                                nc.vector.tensor_tensor(
                                    out=ao[:], in0=at[:], in1=bzs[:],
                                    op=AOP.mult)
                                for j in range(c.QCH // 128):
                                    slot = qc * (c.QCH // 128) + j
                                    nc.sync.dma_start(
                                        out=a2a_in[b][slot][
                                            h * 128:(h + 1) * 128, :],
                                        in_=ao[:, j * 128:(j + 1) * 128])

                        # per-batch collective, overlaps next batch's compute
                        nc.gpsimd.collective_compute(
                            "AllToAll", AOP.bypass,
                            replica_groups=[list(range(c.NCORES))],
                            ins=[a2a_in[b].ap().opt()],
                            outs=[a2a_out[b].ap().opt()],
                        )

            # ============ phase 2: output projection (token-sharded) ============
            with tc.tile_pool(name="gath", bufs=1) as gathp, \
                 tc.tile_pool(name="p2", bufs=1) as p2p, \
                 tc.tile_pool(name="wpsum", bufs=2, space="PSUM") as wpsum:
                gaths = []
                for b in range(c.B):
                    g = gathp.tile([128, NGP, 128], BF, tag=f"gath{b}")
                    nc.sync.dma_start(
                        g[:],
                        a2a_out[b].ap().rearrange(
                            "r (g p) t -> p (r g) t", p=128))
                    gaths.append(g)
                def wo_gemm(oc, b, panel):
                    ops = wpsum.tile([128, 512], F32, tag="wo")
                    for ct in range(NGP):
                        nc.tensor.matmul(
                            ops[:], lhsT=gaths[b][:, ct, :],
                            rhs=panel[:, ct, :],
                            start=(ct == 0), stop=(ct == NGP - 1))
                    osb = sbuf.tile([128, 512], BF, tag="osb", bufs=2)
                    nc.scalar.copy(out=osb[:], in_=ops[:])
                    nc.sync.dma_start(
                        out=out_d[b * 128:(b + 1) * 128,
                                  oc * 512:(oc + 1) * 512],
                        in_=osb[:])

                # The last batch's GEMMs wait on its collective, which is
                # delayed by cross-core skew; push the first panels' b3 GEMMs
                # to the very end (cheap re-dequant) so nothing stalls on it.
                DEFER = 3
                NOC = c.D // 512
                tail = []
                for oc in range(NOC + DEFER):
                    src_oc = oc if oc < NOC else oc - NOC
                    panel = p2p.tile([128, NGP, 512], BF, tag="wop", bufs=2)
                    dequant_t(p2p, panel, w_o.ap()[:, src_oc],
                              s_o.ap()[:, src_oc], NGP, 512,
                              chunks=8 if oc == 0 else 4)
                    if oc < NOC:
                        nb = c.B - 1 if oc < DEFER else c.B
                        for b in range(nb):
                            wo_gemm(oc, b, panel)
                        tail.append(None)
                    else:
                        wo_gemm(src_oc, c.B - 1, panel)

    nc.compile()
    return nc


# ---------------- host-side input prep ----------------

def prep_core_inputs(cfg: Cfg, x, cos_half, sin_half, mask,
                     wq_w, wq_s, wk_w, wk_s, wv_w, wv_s, wo_w, wo_s):
    """Build in_maps (list of dicts, one per core) from full inputs."""
    import ml_dtypes
    c = cfg
    bf16 = ml_dtypes.bfloat16
    HD2 = 64
    NGP = c.NGP
    OSH = c.C_SHARD
    TPB = c.S // 128

    # x retiled: [p=i%128, tile, g=i//128, t']
    x5 = np.ascontiguousarray(
        np.asarray(x).reshape(c.T // 128, 128, NGP, 128).transpose(3, 0, 2, 1)
    ).astype(bf16, copy=False)

    # rope tables [128, TPB, 128], compact (broadcast over heads on-chip)
    ch = np.asarray(cos_half, np.float32)  # [S, 64]
    sh = np.asarray(sin_half, np.float32)
    cos = np.concatenate([ch, ch], axis=1).astype(bf16).astype(np.float32)
    sin = np.concatenate([sh, sh], axis=1).astype(bf16).astype(np.float32)
    sins = sin.copy()
    sins[:, :HD2] = -sin[:, :HD2]
    cosc = np.ascontiguousarray(
        cos.reshape(TPB, 128, 128).transpose(1, 0, 2)).astype(bf16)
    sinc = np.ascontiguousarray(
        sins.reshape(TPB, 128, 128).transpose(1, 0, 2)).astype(bf16)

    # diagonal 0/1 mask block: maskd[k, q] = 1 where mask[q, k] == 0
    m = np.asarray(mask, np.float32)[:128, :128]
    maskd = (m.T == 0.0).astype(np.float32).astype(bf16)

    def pack_w(pw, o_n, panel=None):
        """packed [o_n*NGP, 64] -> unpacked int4 values [128, NGP, o_n]
        with w[p, g, o] = W_q[o, 128*g + p] (or panel-major 4D)."""
        a = np.asarray(pw).reshape(o_n, NGP, 64)
        msb = (a >> 4).astype(np.int8)                    # i = 128g + f
        lsb = (((a & 15) ^ 8) - 8).astype(np.int8)        # i = 128g + 64 + f
        full = np.concatenate(
            [msb.transpose(2, 1, 0), lsb.transpose(2, 1, 0)], axis=0)
        if panel is None:
            return np.ascontiguousarray(full)             # [128, NGP, o_n]
        full = full.reshape(128, NGP, panel, o_n // panel)
        return np.ascontiguousarray(full.transpose(0, 2, 1, 3))

    def pack_s(ps, o_n, panel=None):
        """scales [o_n*2*NGP, 1] -> host-expanded [128, NGP, o_n]
        (rows 0:64 msb scale, 64:128 lsb scale), or panel-major 4D."""
        a = np.asarray(ps).astype(np.float32).reshape(o_n, NGP, 2)
        two = a.transpose(2, 1, 0)  # [2, NGP, o_n]
        full = np.concatenate([
            np.broadcast_to(two[0:1], (64, NGP, o_n)),
            np.broadcast_to(two[1:2], (64, NGP, o_n))], axis=0)
        if panel is None:
            return np.ascontiguousarray(full).astype(bf16)
        full = full.reshape(128, NGP, panel, o_n // panel)
        return np.ascontiguousarray(full.transpose(0, 2, 1, 3)).astype(bf16)

    wo_bt = pack_w(wo_w, c.D, panel=c.D // 512)
    wo_sc = pack_s(wo_s, c.D, panel=c.D // 512)

    in_maps = []
    for core in range(c.NCORES):
        r0 = core * OSH * NGP
        g0 = core * OSH * 2 * NGP
        in_maps.append({
            "x": x5,
            "wq_w": pack_w(np.asarray(wq_w)[r0:r0 + OSH * NGP], OSH),
            "wq_s": pack_s(np.asarray(wq_s)[g0:g0 + OSH * 2 * NGP], OSH),
            "wk_w": pack_w(np.asarray(wk_w)[r0:r0 + OSH * NGP], OSH),
            "wk_s": pack_s(np.asarray(wk_s)[g0:g0 + OSH * 2 * NGP], OSH),
            "wv_w": pack_w(np.asarray(wv_w)[r0:r0 + OSH * NGP], OSH),
            "wv_s": pack_s(np.asarray(wv_s)[g0:g0 + OSH * 2 * NGP], OSH),
            "wo_w": wo_bt,
            "wo_s": wo_sc,
            "cosc": cosc,
            "sinc": sinc,
            "maskd": maskd,
        })
    return in_maps


def unshard_output(cfg: Cfg, results):
    """results: list per core of {"out": [TPC, D]}. Returns [B, S, D].

    Core j's output rows b*128:(b+1)*128 hold global token tile 8*b + j."""
    c = cfg
    TPB = c.S // 128
    full = np.empty((c.B * TPB, 128, c.D),
                    dtype=np.asarray(results[0]["out"]).dtype)
    for j in range(c.NCORES):
        o = np.asarray(results[j]["out"]).reshape(c.B, 128, c.D)
        for b in range(c.B):
            full[TPB * b + j] = o[b]
    return full.reshape(c.B, c.S, c.D)


# ======================================================================
# Self-contained kernel entry point.
# ======================================================================

_CACHE = {}


def _get_program(cfg):
    key = (cfg.B, cfg.S, cfg.D, cfg.NCORES, cfg.SCH, cfg.QCH)
    if key not in _CACHE:
        _CACHE[key] = build_program(cfg)
    return _CACHE[key]


def kernel(x, start_pos=0, cos_half=None, sin_half=None, mask=None,
           wq_w=None, wq_s=None, wk_w=None, wk_s=None,
           wv_w=None, wv_s=None, wo_w=None, wo_s=None,
           cache_k_w=None, cache_k_s=None, cache_v_w=None, cache_v_s=None,
           **_unused):
    from concourse.bass_utils import run_bass_kernel_spmd

    assert int(start_pos) == 0, "kernel specialised for start_pos == 0"
    x = np.asarray(x)
    B, S, D = x.shape
    cfg = Cfg(B=B, S=S, D=D, NCORES=8, SCH=512, QCH=512)
    # start_pos==0 with S==MAX_S, B==MAX_B: the quantized KV cache is fully
    # overwritten before use, so cache_* inputs cannot affect the output.
    in_maps = prep_core_inputs(cfg, x, cos_half, sin_half, mask,
                               wq_w, wq_s, wk_w, wk_s, wv_w, wv_s,
                               wo_w, wo_s)
    nc = _get_program(cfg)
    res = run_bass_kernel_spmd(nc, in_maps, core_ids=list(range(cfg.NCORES)))
    out = unshard_output(cfg, res.results)
    import ml_dtypes
    return out.astype(ml_dtypes.bfloat16, copy=False)


# revision 49
# speedup vs baseline: 1.0475x; 1.0181x over previous
"""Trainium2 (Bass/Tile) kernel for quantized multi-head attention.

Distributed across 8 NeuronCores: tensor-parallel over heads for the
QKV projections + RoPE + causal attention, per-batch AllToAll
collectives (overlapped with later batches' compute), then a
token-parallel output projection over interleaved 128-token tiles.

The Q4_0 weights ship host-UNPACKED (int4 values widened to int8, laid
out transposed [in%128, in//128, out] so dequant lands directly in the
matmul-rhs layout with no PE transposes) alongside host-expanded group
scales; on-chip dequant is a single chunked tensor_tensor multiply (DVE
for QKV at startup, GpSimd for the wo panels so they overlap the last
batch's attention). The causal mask is a 0/1 multiply after exp, and
softmax normalization divides after the PV matmul (linearity), with the
1/z partition-broadcast on GpSimd.
"""

import math
from dataclasses import dataclass

import numpy as np

import concourse.bass as bass
import concourse.tile as tile
from concourse.masks import make_identity
from concourse import bacc, mybir

BF = mybir.dt.bfloat16
F32 = mybir.dt.float32
I8 = mybir.dt.int8
AOP = mybir.AluOpType
AF = mybir.ActivationFunctionType


@dataclass
class Cfg:
    B: int = 4
    S: int = 1024
    D: int = 4096
    NCORES: int = 8
    SCH: int = 512   # kept for test.py compat (unused)
    QCH: int = 512   # attention q-chunk

    @property
    def T(self):
        return self.B * self.S

    @property
    def H(self):
        return self.D // 128  # total heads (head_dim 128)

    @property
    def H_LOC(self):
        return self.H // self.NCORES

    @property
    def C_SHARD(self):
        return self.H_LOC * 128  # local channels

    @property
    def TPC(self):
        return self.T // self.NCORES  # tokens per core (output slice)

    @property
    def NGP(self):
        return self.D // 128  # contraction k-tiles per row


def build_program(cfg: Cfg):
    """Build the per-core Bass program. Returns compiled nc."""
    c = cfg
    assert c.QCH == 512 and c.S == 1024 and c.NCORES == 8

    import concourse.tile_utils as tile_utils
    tile_utils.max_sbuf_usage = 208 * 1024

    nc = bacc.Bacc("TRN2", target_bir_lowering=False, debug=False,
                   num_devices=c.NCORES)

    OSH = c.C_SHARD          # qkv weight shard out-channels per core (512)
    NGP = c.NGP              # 32
    NTIL = c.T // 128        # 32 global token tiles
    TPB = c.S // 128         # 8 tiles per batch

    # ---- external I/O ----
    # x retiled: [p=i%128, tile, g=i//128, t']
    x_d = nc.dram_tensor("x", [128, NTIL, NGP, 128], BF, kind="ExternalInput")
    # unpacked int4 values, transposed: wt[p=i%128, g=i//128, o]
    w_q = nc.dram_tensor("wq_w", [128, NGP, OSH], I8, kind="ExternalInput")
    s_q = nc.dram_tensor("wq_s", [128, NGP, OSH], BF, kind="ExternalInput")
    w_k = nc.dram_tensor("wk_w", [128, NGP, OSH], I8, kind="ExternalInput")
    s_k = nc.dram_tensor("wk_s", [128, NGP, OSH], BF, kind="ExternalInput")
    w_v = nc.dram_tensor("wv_w", [128, NGP, OSH], I8, kind="ExternalInput")
    s_v = nc.dram_tensor("wv_s", [128, NGP, OSH], BF, kind="ExternalInput")
    # wo panel-major: [p, oc, g, o']
    w_o = nc.dram_tensor("wo_w", [128, c.D // 512, NGP, 512], I8,
                         kind="ExternalInput")
    s_o = nc.dram_tensor("wo_s", [128, c.D // 512, NGP, 512], BF,
                         kind="ExternalInput")
    # rope tables, compact: [p=s%128, ssub=s//128, d]
    cosc_d = nc.dram_tensor("cosc", [128, TPB, 128], BF, kind="ExternalInput")
    sinc_d = nc.dram_tensor("sinc", [128, TPB, 128], BF, kind="ExternalInput")
    maskd_d = nc.dram_tensor("maskd", [128, 128], BF, kind="ExternalInput")
    out_d = nc.dram_tensor("out", [c.TPC, c.D], BF, kind="ExternalOutput")

    # per-batch collective bounce buffers; slot j = within-batch token tile j
    a2a_in = [nc.dram_tensor(f"a2a_in{b}", [c.NCORES, c.C_SHARD, 128], BF)
              for b in range(c.B)]
    a2a_out = [nc.dram_tensor(f"a2a_out{b}", [c.NCORES, c.C_SHARD, 128], BF)
               for b in range(c.B)]

    inv_sqrt_d = 1.0 / math.sqrt(128.0)

    def dequant_t(pool, wt, bt_ap, sc_ap, ngp, osz, chunks=4, eng=None):
        """Dequantize unpacked int4 values into transposed wt [128, ngp, osz].

        bt_ap: DRAM [128, ngp, osz] int8 values; sc_ap: DRAM [128, ngp, osz]
        host-expanded scales. Works in double-buffered [128, ngp/chunks, osz]
        chunk tiles so DMAs pipeline and consumers can start early."""
        if eng is None:
            eng = nc.vector
        gch = ngp // chunks
        for i in range(chunks):
            g0 = i * gch
            nq = pool.tile([128, gch, osz], I8, tag="dq_nb", bufs=2)
            sc = pool.tile([128, gch, osz], BF, tag="dq_sc", bufs=2)
            nc.sync.dma_start(nq[:], bt_ap[:, g0:g0 + gch, :])
            nc.sync.dma_start(sc[:], sc_ap[:, g0:g0 + gch, :])
            eng.tensor_tensor(
                out=wt[:, g0:g0 + gch, :], in0=nq[:], in1=sc[:],
                op=AOP.mult)

    with tile.TileContext(nc) as tc:
        with tc.tile_pool(name="const", bufs=1) as const, \
             tc.tile_pool(name="sbuf", bufs=2) as sbuf:
            # constants
            cosc = const.tile([128, TPB, 128], BF)
            nc.sync.dma_start(cosc[:], cosc_d[:])
            sinc = const.tile([128, TPB, 128], BF)
            nc.sync.dma_start(sinc[:], sinc_d[:])
            maskd = const.tile([128, 128], BF)
            nc.sync.dma_start(maskd[:], maskd_d[:])
            ones_col = const.tile([128, 1], BF)
            nc.vector.memset(ones_col[:], 1.0)
            ones_row = const.tile([1, 128], BF)
            nc.vector.memset(ones_row[:], 1.0)
            ident = const.tile([128, 128], BF)
            make_identity(nc, ident)

            # ============ phase 1: QKV + attention ============
            with tc.tile_pool(name="wt", bufs=1) as wtp:
                wt_q = wtp.tile([128, NGP, OSH], BF, tag="wt_q")
                wt_k = wtp.tile([128, NGP, OSH], BF, tag="wt_k")
                wt_v = wtp.tile([128, NGP, OSH], BF, tag="wt_v")
                with tc.tile_pool(name="dqp", bufs=1) as dqp, \
                     tc.tile_pool(name="xt", bufs=1) as xtp, \
                     tc.tile_pool(name="kqv", bufs=2) as kqvp, \
                     tc.tile_pool(name="pt", bufs=4) as ptp, \
                     tc.tile_pool(name="ppsum", bufs=2, space="PSUM") as ppsum, \
                     tc.tile_pool(name="spsum", bufs=2, space="PSUM") as spsum, \
                     tc.tile_pool(name="zpsum", bufs=1, space="PSUM") as zpsum, \
                     tc.tile_pool(name="apsum", bufs=1, space="PSUM") as apsum, \
                     tc.tile_pool(name="tpsum", bufs=2, space="PSUM") as tpsum:

                    def proj_one(mat, wt_m, xt_ts, ts, kt_b, qt_b, v_b):
                        st0 = ts * 128
                        ps = ppsum.tile([128, OSH], F32, tag="proj")
                        for gp in range(NGP):
                            nc.tensor.matmul(
                                ps[:],
                                lhsT=xt_ts[:, gp, :],
                                rhs=wt_m[:, gp, :],
                                start=(gp == 0),
                                stop=(gp == NGP - 1))
                        if mat == "v":
                            nc.scalar.copy(out=v_b[:, ts, :], in_=ps[:])
                            return
                        # single PSUM read, then rope from SBUF bf16
                        psc = sbuf.tile([128, c.C_SHARD], BF,
                                        tag="psc", bufs=2)
                        nc.scalar.copy(out=psc[:], in_=ps[:])
                        # rope: roped = psc*cos + swaphalf(psc)*sin(+/-)
                        roped = sbuf.tile([128, c.C_SHARD], BF,
                                          tag="roped", bufs=2)
                        tmp = sbuf.tile([128, c.C_SHARD], BF,
                                        tag="ropetmp", bufs=2)
                        p3 = psc[:].rearrange("p (h d) -> p h d", d=128)
                        t3 = tmp[:].rearrange("p (h d) -> p h d", d=128)
                        r3 = roped[:].rearrange("p (h d) -> p h d", d=128)
                        c3 = cosc[:, ts, :][:, None, :].to_broadcast(
                            [128, c.H_LOC, 128])
                        s3 = sinc[:, ts, :][:, None, :].to_broadcast(
                            [128, c.H_LOC, 128])
                        nc.vector.tensor_tensor(
                            out=t3[:, :, 0:64], in0=p3[:, :, 64:128],
                            in1=s3[:, :, 0:64], op=AOP.mult)
                        nc.vector.tensor_tensor(
                            out=t3[:, :, 64:128], in0=p3[:, :, 0:64],
                            in1=s3[:, :, 64:128], op=AOP.mult)
                        nc.vector.tensor_tensor(
                            out=r3[:], in0=p3[:], in1=c3, op=AOP.mult)
                        nc.vector.tensor_tensor(
                            out=roped[:], in0=roped[:], in1=tmp[:],
                            op=AOP.add)
                        dst = qt_b if mat == "q" else kt_b
                        for h in range(c.H_LOC):
                            tp = tpsum.tile([128, 128], BF, tag="tp",
                                            bufs=2)
                            nc.tensor.transpose(
                                tp[:], roped[:, h * 128:(h + 1) * 128],
                                ident[:])
                            if h % 2 == 0:
                                nc.scalar.copy(
                                    out=dst[:, h, st0:st0 + 128], in_=tp[:])
                            else:
                                nc.vector.tensor_copy(
                                    out=dst[:, h, st0:st0 + 128], in_=tp[:])

                    dq_srcs = {"q": (wt_q, w_q, s_q), "k": (wt_k, w_k, s_k),
                               "v": (wt_v, w_v, s_v)}
                    for b in range(c.B):
                        # per-batch K/Q transposed and V natural
                        kt_b = kqvp.tile([128, c.H_LOC, c.S], BF, tag="kt_b")
                        qt_b = kqvp.tile([128, c.H_LOC, c.S], BF, tag="qt_b")
                        v_b = kqvp.tile([128, TPB, c.C_SHARD], BF, tag="v_b",
                                        bufs=1)
                        if b == 0:
                            # mat-outer, dequant interleaved: q projections
                            # start right after wq's dequant while k/v still
                            # dequantize (x tiles re-loaded per mat)
                            for mat, wt_m in (("q", wt_q), ("k", wt_k),
                                              ("v", wt_v)):
                                _, w_m, s_m = dq_srcs[mat]
                                dequant_t(dqp, wt_m, w_m.ap(), s_m.ap(),
                                          NGP, OSH, chunks=8)
                                for ts in range(TPB):
                                    xt_ts = xtp.tile([128, NGP, 128], BF,
                                                     tag="xt", bufs=2)
                                    nc.sync.dma_start(xt_ts[:],
                                                      x_d.ap()[:, ts])
                                    proj_one(mat, wt_m, xt_ts, ts,
                                             kt_b, qt_b, v_b)
                        else:
                            for ts in range(TPB):
                                tt = b * TPB + ts
                                xt_ts = xtp.tile([128, NGP, 128], BF,
                                                 tag="xt", bufs=2)
                                nc.sync.dma_start(xt_ts[:], x_d.ap()[:, tt])
                                for mat, wt_m in (("q", wt_q), ("k", wt_k),
                                                  ("v", wt_v)):
                                    proj_one(mat, wt_m, xt_ts, ts,
                                             kt_b, qt_b, v_b)

                        # ---- attention for batch b ----
                        for h in range(c.H_LOC):
                            for qc in range(c.S // c.QCH):
                                q0 = qc * c.QCH
                                kmax = (q0 + c.QCH) // 128
                                at = apsum.tile([128, c.QCH], F32, tag="at")
                                zp = zpsum.tile([1, c.QCH], F32, tag="z")
                                psum_tree = sbuf.tile([128, c.QCH], BF,
                                                      tag="ptree", bufs=2)
                                for ki in range(kmax):
                                    off = max(0, 128 * ki - q0)
                                    stp = spsum.tile([128, c.QCH], F32,
                                                     tag="sc")
                                    nc.tensor.matmul(
                                        stp[:, off:],
                                        lhsT=kt_b[:, h,
                                                  ki * 128:(ki + 1) * 128],
                                        rhs=qt_b[:, h, q0 + off:q0 + c.QCH],
                                        start=True, stop=True)
                                    pt = ptp.tile([128, c.QCH], BF, tag="pt")
                                    nc.scalar.activation(
                                        out=pt[:, off:], in_=stp[:, off:],
                                        func=AF.Exp, scale=inv_sqrt_d)
                                    if 128 * ki >= q0:
                                        # zero the upper triangle of the
                                        # diagonal block (causal mask)
                                        nc.vector.tensor_tensor(
                                            out=pt[:, off:off + 128],
                                            in0=pt[:, off:off + 128],
                                            in1=maskd[:], op=AOP.mult)
                                    if ki == 0:
                                        nc.vector.tensor_copy(
                                            out=psum_tree[:], in_=pt[:])
                                    else:
                                        nc.vector.tensor_tensor(
                                            out=psum_tree[:, off:],
                                            in0=psum_tree[:, off:],
                                            in1=pt[:, off:], op=AOP.add)
                                    nc.tensor.matmul(
                                        at[:, off:],
                                        lhsT=v_b[:, ki,
                                                 h * 128:(h + 1) * 128],
                                        rhs=pt[:, off:],
                                        start=(ki == 0),
                                        stop=(ki == kmax - 1))
                                nc.tensor.matmul(
                                    zp[:], lhsT=ones_col[:], rhs=psum_tree[:],
                                    start=True, stop=True)
                                rz = sbuf.tile([1, c.QCH], F32, tag="rz")
                                nc.vector.reciprocal_approx_fast(rz[:], zp[:])
                                bzs = sbuf.tile([128, c.QCH], F32, tag="bzs")
                                nc.gpsimd.partition_broadcast(bzs[:], rz[:])
                                ao = sbuf.tile([128, c.QCH], BF, tag="ao")
                                nc.vector.tensor_tensor(
                                    out=ao[:], in0=at[:], in1=bzs[:],
                                    op=AOP.mult)
                                for j in range(c.QCH // 128):
                                    slot = qc * (c.QCH // 128) + j
                                    nc.sync.dma_start(
                                        out=a2a_in[b][slot][
                                            h * 128:(h + 1) * 128, :],
                                        in_=ao[:, j * 128:(j + 1) * 128])

                        # per-batch collective, overlaps next batch's compute
                        nc.gpsimd.collective_compute(
                            "AllToAll", AOP.bypass,
                            replica_groups=[list(range(c.NCORES))],
                            ins=[a2a_in[b].ap().opt()],
                            outs=[a2a_out[b].ap().opt()],
                        )

            # ============ phase 2: output projection (token-sharded) ============
            with tc.tile_pool(name="gath", bufs=1) as gathp, \
                 tc.tile_pool(name="p2", bufs=1) as p2p, \
                 tc.tile_pool(name="wpsum", bufs=2, space="PSUM") as wpsum:
                gaths = []
                for b in range(c.B):
                    g = gathp.tile([128, NGP, 128], BF, tag=f"gath{b}")
                    nc.sync.dma_start(
                        g[:],
                        a2a_out[b].ap().rearrange(
                            "r (g p) t -> p (r g) t", p=128))
                    gaths.append(g)
                def wo_gemm(oc, b, panel):
                    ops = wpsum.tile([128, 512], F32, tag="wo")
                    for ct in range(NGP):
                        nc.tensor.matmul(
                            ops[:], lhsT=gaths[b][:, ct, :],
                            rhs=panel[:, ct, :],
                            start=(ct == 0), stop=(ct == NGP - 1))
                    osb = sbuf.tile([128, 512], BF, tag="osb", bufs=2)
                    nc.scalar.copy(out=osb[:], in_=ops[:])
                    nc.sync.dma_start(
                        out=out_d[b * 128:(b + 1) * 128,
                                  oc * 512:(oc + 1) * 512],
                        in_=osb[:])

                # The last batch's GEMMs wait on its collective, which is
                # delayed by cross-core skew; push the first panels' b3 GEMMs
                # to the very end (cheap re-dequant) so nothing stalls on it.
                DEFER = 2
                NOC = c.D // 512
                tail = []
                for oc in range(NOC + DEFER):
                    src_oc = oc if oc < NOC else oc - NOC
                    panel = p2p.tile([128, NGP, 512], BF, tag="wop", bufs=2)
                    dequant_t(p2p, panel, w_o.ap()[:, src_oc],
                              s_o.ap()[:, src_oc], NGP, 512,
                              chunks=8 if oc == 0 else 4)
                    if oc < NOC:
                        nb = c.B - 1 if oc < DEFER else c.B
                        for b in range(nb):
                            wo_gemm(oc, b, panel)
                        tail.append(None)
                    else:
                        wo_gemm(src_oc, c.B - 1, panel)

    nc.compile()
    return nc


# ---------------- host-side input prep ----------------

def prep_core_inputs(cfg: Cfg, x, cos_half, sin_half, mask,
                     wq_w, wq_s, wk_w, wk_s, wv_w, wv_s, wo_w, wo_s):
    """Build in_maps (list of dicts, one per core) from full inputs."""
    import ml_dtypes
    c = cfg
    bf16 = ml_dtypes.bfloat16
    HD2 = 64
    NGP = c.NGP
    OSH = c.C_SHARD
    TPB = c.S // 128

    # x retiled: [p=i%128, tile, g=i//128, t']
    x5 = np.ascontiguousarray(
        np.asarray(x).reshape(c.T // 128, 128, NGP, 128).transpose(3, 0, 2, 1)
    ).astype(bf16, copy=False)

    # rope tables [128, TPB, 128], compact (broadcast over heads on-chip)
    ch = np.asarray(cos_half, np.float32)  # [S, 64]
    sh = np.asarray(sin_half, np.float32)
    cos = np.concatenate([ch, ch], axis=1).astype(bf16).astype(np.float32)
    sin = np.concatenate([sh, sh], axis=1).astype(bf16).astype(np.float32)
    sins = sin.copy()
    sins[:, :HD2] = -sin[:, :HD2]
    cosc = np.ascontiguousarray(
        cos.reshape(TPB, 128, 128).transpose(1, 0, 2)).astype(bf16)
    sinc = np.ascontiguousarray(
        sins.reshape(TPB, 128, 128).transpose(1, 0, 2)).astype(bf16)

    # diagonal 0/1 mask block: maskd[k, q] = 1 where mask[q, k] == 0
    m = np.asarray(mask, np.float32)[:128, :128]
    maskd = (m.T == 0.0).astype(np.float32).astype(bf16)

    def pack_w(pw, o_n, panel=None):
        """packed [o_n*NGP, 64] -> unpacked int4 values [128, NGP, o_n]
        with w[p, g, o] = W_q[o, 128*g + p] (or panel-major 4D)."""
        a = np.asarray(pw).reshape(o_n, NGP, 64)
        msb = (a >> 4).astype(np.int8)                    # i = 128g + f
        lsb = (((a & 15) ^ 8) - 8).astype(np.int8)        # i = 128g + 64 + f
        full = np.concatenate(
            [msb.transpose(2, 1, 0), lsb.transpose(2, 1, 0)], axis=0)
        if panel is None:
            return np.ascontiguousarray(full)             # [128, NGP, o_n]
        full = full.reshape(128, NGP, panel, o_n // panel)
        return np.ascontiguousarray(full.transpose(0, 2, 1, 3))

    def pack_s(ps, o_n, panel=None):
        """scales [o_n*2*NGP, 1] -> host-expanded [128, NGP, o_n]
        (rows 0:64 msb scale, 64:128 lsb scale), or panel-major 4D."""
        a = np.asarray(ps).astype(np.float32).reshape(o_n, NGP, 2)
        two = a.transpose(2, 1, 0)  # [2, NGP, o_n]
        full = np.concatenate([
            np.broadcast_to(two[0:1], (64, NGP, o_n)),
            np.broadcast_to(two[1:2], (64, NGP, o_n))], axis=0)
        if panel is None:
            return np.ascontiguousarray(full).astype(bf16)
        full = full.reshape(128, NGP, panel, o_n // panel)
        return np.ascontiguousarray(full.transpose(0, 2, 1, 3)).astype(bf16)

    wo_bt = pack_w(wo_w, c.D, panel=c.D // 512)
    wo_sc = pack_s(wo_s, c.D, panel=c.D // 512)

    in_maps = []
    for core in range(c.NCORES):
        r0 = core * OSH * NGP
        g0 = core * OSH * 2 * NGP
        in_maps.append({
            "x": x5,
            "wq_w": pack_w(np.asarray(wq_w)[r0:r0 + OSH * NGP], OSH),
            "wq_s": pack_s(np.asarray(wq_s)[g0:g0 + OSH * 2 * NGP], OSH),
            "wk_w": pack_w(np.asarray(wk_w)[r0:r0 + OSH * NGP], OSH),
            "wk_s": pack_s(np.asarray(wk_s)[g0:g0 + OSH * 2 * NGP], OSH),
            "wv_w": pack_w(np.asarray(wv_w)[r0:r0 + OSH * NGP], OSH),
            "wv_s": pack_s(np.asarray(wv_s)[g0:g0 + OSH * 2 * NGP], OSH),
            "wo_w": wo_bt,
            "wo_s": wo_sc,
            "cosc": cosc,
            "sinc": sinc,
            "maskd": maskd,
        })
    return in_maps


def unshard_output(cfg: Cfg, results):
    """results: list per core of {"out": [TPC, D]}. Returns [B, S, D].

    Core j's output rows b*128:(b+1)*128 hold global token tile 8*b + j."""
    c = cfg
    TPB = c.S // 128
    full = np.empty((c.B * TPB, 128, c.D),
                    dtype=np.asarray(results[0]["out"]).dtype)
    for j in range(c.NCORES):
        o = np.asarray(results[j]["out"]).reshape(c.B, 128, c.D)
        for b in range(c.B):
            full[TPB * b + j] = o[b]
    return full.reshape(c.B, c.S, c.D)


# ======================================================================
# Self-contained kernel entry point.
# ======================================================================

_CACHE = {}


def _get_program(cfg):
    key = (cfg.B, cfg.S, cfg.D, cfg.NCORES, cfg.SCH, cfg.QCH)
    if key not in _CACHE:
        _CACHE[key] = build_program(cfg)
    return _CACHE[key]


def kernel(x, start_pos=0, cos_half=None, sin_half=None, mask=None,
           wq_w=None, wq_s=None, wk_w=None, wk_s=None,
           wv_w=None, wv_s=None, wo_w=None, wo_s=None,
           cache_k_w=None, cache_k_s=None, cache_v_w=None, cache_v_s=None,
           **_unused):
    from concourse.bass_utils import run_bass_kernel_spmd

    assert int(start_pos) == 0, "kernel specialised for start_pos == 0"
    x = np.asarray(x)
    B, S, D = x.shape
    cfg = Cfg(B=B, S=S, D=D, NCORES=8, SCH=512, QCH=512)
    # start_pos==0 with S==MAX_S, B==MAX_B: the quantized KV cache is fully
    # overwritten before use, so cache_* inputs cannot affect the output.
    in_maps = prep_core_inputs(cfg, x, cos_half, sin_half, mask,
                               wq_w, wq_s, wk_w, wk_s, wv_w, wv_s,
                               wo_w, wo_s)
    nc = _get_program(cfg)
    res = run_bass_kernel_spmd(nc, in_maps, core_ids=list(range(cfg.NCORES)))
    out = unshard_output(cfg, res.results)
    import ml_dtypes
    return out.astype(ml_dtypes.bfloat16, copy=False)
